# revision 9
# baseline (speedup 1.0000x reference)
"""AWPINN wavelet-PINN kernel for 8x Trainium2 NeuronCores (Bass/Tile).

Math: for each point i and wavelet k (N=65536, K=512):
  xt = wx*x - bx (same y,z);  s = xt^2+yt^2+zt^2;  E = exp(-0.5*s)
  W  = xt*yt*zt*E          (reference's xw*yw*zw = -W)
  output = sum_k (-coeff*scale)_k * W + bias
  d2u_dx2 = sum_k (coeff*scale*wx^2)_k * (3 - xt^2) * W   (same y,z)

Device structure (v3):
  - s and T3=xt*yt*zt are low-rank bilinear forms in per-point features
    F = [x2,y2,z2,xyz,xy,xz,yz,x,y,z,1] -> TensorEngine matmuls
    (contraction = features, M = 128 wavelets/block, FD = 512 points).
    All matmuls fp16; near-fp32 via hi/lo split stacked along contraction
    ([Lh;Ll;Lh10] @ [Fh;Fh;Fl10]). Feature stack replicated host-side to
    [128, NP] so each slice loads with ONE dma into all 4 row groups.
  - The PE clock ramps to 2.4 GHz only after ~3.4us of GAPLESS matmul
    activity and re-throttles to 1.2 GHz after idle gaps. A 9-matmul
    garbage prewarm burst ramps it during the input DMAs, and filler
    matmuls (into scratch psum rows 64-73 of the output bank) are
    interleaved with the real work so the PE queue never goes empty.
  - Per chunk (512 pts): 2 k-pair groups of 4 row-tiled feature MMs
    (s halves -> 2-bank psum pair, T3 -> two 1-bank psum tiles), one
    [128,1024] Exp on ACT, two [128,512] W=T3*E muls on DVE; then 4
    output MMs (hi-only lhsT [128,10]; fp16 rounding of the output
    weights costs ~1e-3 rel err, gate is 2e-2) accumulate back-to-back
    into rows 0-9 (even chunk) / 32-41 (odd chunk) of a [74,512] psum
    tile, so ONE [42,512] ACT copy per 2 chunks drains both, deferred
    into the next chunk to stay off the EXP critical path.
  - PSUM: s-pairs 2x2 banks + T3 2x1 + out/filler [74,512] 2x1 = 8.
  - R transposes to point-major via direct SBUF->SBUF dma per drain (no
    DRAM bounce); the output row goes to DRAM straight from r_rows.
  - Epilogue: 3 phases (partitions 0:64 / 64:96 / 96:128) rebuild d2
    from R; first two run on idle GPSIMD during the main loop, the last
    on DVE in the tail. Chunks run in order [0-7,12,13,8-11,14,15] so
    the last phase only waits on chunks 14/15.
Data parallel over points: each core handles 8192 points; no collectives.
"""

import numpy as np

N_TOTAL = 65536
K_TOTAL = 512
N_CORES = 8
NP_CORE = N_TOTAL // N_CORES        # 8192 points per core
CHUNK = 512                         # points per matmul (PSUM bank = 512 fp32)
N_CHUNKS = NP_CORE // CHUNK         # 16
KBLK = K_TOTAL // 128               # 4 wavelet blocks of 128
EPP = NP_CORE // 128                # 64 = free dim of [128, 64] point layout
NFEAT = 11                          # features per point
N_PREWARM = 9                       # garbage matmuls to ramp the PE p-state
# processing order: phase boundaries need partition bases 0/32/64/96, and
# the final phase (chunks 12-15) should have 12/13 done early
CHUNK_ORDER = [0, 1, 2, 3, 4, 5, 6, 7, 12, 13, 8, 9, 10, 11, 14, 15]

_COMPILED = {}


def _split16(a):
    """Split fp32 into fp16 hi + fp16 lo (hi+lo carries ~21 mantissa bits)."""
    a = np.ascontiguousarray(a, np.float32)
    hi = a.astype(np.float16)
    lo = np.float32(a - hi.astype(np.float32)).astype(np.float16)
    return hi, lo


def _stack32(L):
    """[11,n] fp32 coeffs -> [32,n] fp16 stack [Lh; Ll; Lh[:10]]."""
    Lh, Ll = _split16(L)
    return np.concatenate([Lh, Ll, Lh[:NFEAT - 1]], axis=0)


def _build_program():
    import concourse.bacc as bacc
    import concourse.mybir as mybir
    import concourse.tile as tile

    f32 = mybir.dt.float32
    f16 = mybir.dt.float16
    AF = mybir.ActivationFunctionType

    nc = bacc.Bacc("TRN2", target_bir_lowering=False, debug=False)

    fst_d = nc.dram_tensor("fst", [128, NP_CORE], f16, kind="ExternalInput")
    # lst: rows 0-31 Ls-stack, 32-63 Lt-stack, 64-95 Ls, 96-127 Lt;
    # columns grouped by k-block
    lst_d = nc.dram_tensor("lst", [128, K_TOTAL], f16, kind="ExternalInput")
    loh_d = nc.dram_tensor("loh", [128, KBLK * 10], f16, kind="ExternalInput")
    ep_d = nc.dram_tensor("ep", [6, NP_CORE], f32, kind="ExternalInput")
    out_d = nc.dram_tensor("out", [4, NP_CORE], f32, kind="ExternalOutput")

    with tile.TileContext(nc) as tc:
        with (
            tc.tile_pool(name="persist", bufs=1) as pp,
            tc.tile_pool(name="work", bufs=4) as wp,
            tc.tile_pool(name="psum_s", bufs=2, space="PSUM") as psps,
            tc.tile_pool(name="psum_t", bufs=2, space="PSUM") as pspt,
            tc.tile_pool(name="psum_out", bufs=1, space="PSUM") as pso,
            tc.tile_pool(name="wpool", bufs=4) as wpool,
            tc.tile_pool(name="dram", bufs=1, space="DRAM") as dp,
        ):
            # --- prewarm: ramp the PE with garbage matmuls while the input
            # DMAs run (reads an uninitialized tile; values are irrelevant)
            pg = pp.tile([32, 128 + CHUNK], f16, tag="prewarm_src")
            nc.gpsimd.memset(pg[:], 0.0)
            # single persistent [74, 1024] psum tile for output rows +
            # filler scratch: columns double-buffer slot pairs manually,
            # rows 64-73 are a write-only filler target with no readers so
            # filler matmuls never carry cross-engine dependencies
            po_all = pso.tile([74, 2 * CHUNK], f32, tag="po_all")
            pw_ps = psps.tile([128, 2 * CHUNK], f32, tag="ps_s")
            for i in range(N_PREWARM):
                nc.tensor.matmul(
                    pw_ps[:, 0:CHUNK], pg[:, 0:128], pg[:, 128:],
                    start=True, stop=True)

            lst_t = pp.tile([128, K_TOTAL], f16, tag="lst")
            nc.sync.dma_start(lst_t[:], lst_d[:])
            loh_t = pp.tile([128, KBLK * 10], f16, tag="loh")
            nc.scalar.dma_start(loh_t[:], loh_d[:])

            # persistent feature stack; slice 1 rides the scalar queue so
            # the first two land in parallel
            f_all = pp.tile([128, NP_CORE], f16, tag="f_all")
            bounds = [0, CHUNK, 2 * CHUNK] + [
                q * NP_CORE // 8 for q in range(2, 9)]
            for q in range(len(bounds) - 1):
                qs = slice(bounds[q], bounds[q + 1])
                eng = nc.scalar if q == 1 else nc.sync
                eng.dma_start(f_all[:, qs], fst_d[:, qs])

            # R staging: copies of psum out rows land here ([42, 512] per
            # 2-chunk drain at the slot-pair's column); rows 1-9 bounce
            # through DRAM point-major (SBUF dma dst cannot reorder the
            # partition dim), the output row goes to DRAM directly
            r_rows = pp.tile([42, NP_CORE // 2], f32, tag="r_rows")
            rb_full = pp.tile([128, 9 * EPP], f32, tag="rb_full")
            r_dram = dp.tile([128, 9 * EPP], f32, tag="r_dram")

            ep_t = []
            for i in range(6):  # x2, x, y2, y, z2, z
                t = pp.tile([128, EPP], f32, tag=f"ep{i}")
                nc.gpsimd.dma_start(
                    t[:], ep_d[i:i + 1, :].rearrange("o (p f) -> (o p) f", p=128))
                ep_t.append(t)

            pending_drain = []

            def emit_drain():
                if not pending_drain:
                    return
                sc, ch_ev, ch_od, po2 = pending_drain.pop()
                dst = r_rows[:, sc * CHUNK:(sc + 1) * CHUNK]
                nc.scalar.copy(dst, po2[0:42, :])
                for rr, ch in ((0, ch_ev), (32, ch_od)):
                    # output row straight to DRAM; rows 1-9 point-major
                    nc.sync.dma_start(
                        out_d[0:1, ch * CHUNK:(ch + 1) * CHUNK],
                        dst[rr:rr + 1, :])
                    nc.sync.dma_start(
                        r_dram[ch * 8:(ch + 1) * 8, :].rearrange(
                            "p (r f) -> r p f", r=9),
                        dst[rr + 1:rr + 10, :].rearrange(
                            "r (p f) -> r p f", p=8))

            po2 = None
            for slot in range(N_CHUNKS):
                c = CHUNK_ORDER[slot]
                f_t = f_all[:, c * CHUNK:(c + 1) * CHUNK]
                colb = slice(((slot // 2) % 2) * CHUNK,
                             ((slot // 2) % 2 + 1) * CHUNK)
                if slot % 2 == 0:
                    po2 = po_all[:, colb]
                po = po2[32 * (slot % 2):32 * (slot % 2) + 10, :]
                fill = po_all[64:74, 0:CHUNK]

                def filler(n=1):
                    for _ in range(n):
                        nc.tensor.matmul(
                            fill, loh_t[:, 0:10], f_all[:, 0:CHUNK],
                            start=True, stop=True)

                w_ts = []
                for p in range(KBLK // 2):      # k-block pairs
                    kb0, kb1 = 2 * p, 2 * p + 1
                    ps_s = psps.tile([128, 2 * CHUNK], f32, tag="ps_s")
                    pt = [pspt.tile([128, CHUNK], f32, tag="ps_t",
                                    name=f"pt{slot}_{p}_{i}") for i in range(2)]
                    # 4 feature matmuls burst over the four 32-row groups:
                    # g0/g2 = s(kb0/kb1) -> ps_s halves, g1/g3 = T3 -> pt
                    for g, dst, kb in (
                            (0, ps_s[:, 0:CHUNK], kb0),
                            (1, pt[0][:], kb0),
                            (2, ps_s[:, CHUNK:], kb1),
                            (3, pt[1][:], kb1)):
                        nc.tensor.matmul(
                            dst,
                            lst_t[32 * g:32 * (g + 1), kb * 128:(kb + 1) * 128],
                            f_t[32 * g:32 * (g + 1), :],
                            start=True, stop=True,
                            tile_position=(32 * g, 0))
                    filler(1)
                    e_t = wp.tile([128, 2 * CHUNK], f32, tag="e")
                    nc.scalar.activation(e_t[:], ps_s[:], AF.Exp, scale=-0.5)
                    if p == 0 and slot % 2 == 0:
                        emit_drain()   # previous slot-pair's R drain, off
                                       # the critical path of this chunk
                    w_t = wpool.tile([128, 2 * CHUNK], f16, tag="w")
                    nc.vector.tensor_mul(
                        w_t[:, 0:CHUNK], pt[0][:], e_t[:, 0:CHUNK])
                    nc.vector.tensor_mul(
                        w_t[:, CHUNK:], pt[1][:], e_t[:, CHUNK:])
                    w_ts.append(w_t)
                # output matmuls accumulate back-to-back into this chunk's
                # rows; a filler after each keeps the PE queue non-empty
                # while the next W-mul finishes on DVE
                for kb in range(KBLK):
                    w_t = w_ts[kb // 2]
                    half = slice((kb & 1) * CHUNK, ((kb & 1) + 1) * CHUNK)
                    nc.tensor.matmul(
                        po, loh_t[:, kb * 10:(kb + 1) * 10], w_t[:, half],
                        start=(kb == 0), stop=(kb == KBLK - 1))
                    filler(1)
                if slot % 2 == 1:
                    pending_drain.append(
                        (slot // 2, CHUNK_ORDER[slot - 1], c, po2))

                # 3-phase epilogue over point-major partitions
                if slot == 11:
                    # chunks 12/13 (partitions 96-111) drained long ago:
                    # pre-stage their point-major R rows for the last phase
                    nc.sync.dma_start(rb_full[96:112, :], r_dram[96:112, :])
                if slot == 7:
                    hlf, ee = slice(0, 64), nc.gpsimd
                elif slot == 13:
                    hlf, ee = slice(64, 96), nc.gpsimd
                elif slot == 15:
                    hlf, ee = slice(96, 128), nc.vector
                else:
                    continue
                emit_drain()
                npart = hlf.stop - hlf.start
                psl = slice(hlf.start * EPP, hlf.stop * EPP)
                if npart > 32:
                    h0 = slice(hlf.start, hlf.start + npart // 2)
                    h1 = slice(hlf.start + npart // 2, hlf.stop)
                    nc.sync.dma_start(rb_full[h0, :], r_dram[h0, :])
                    nc.sync.dma_start(rb_full[h1, :], r_dram[h1, :])
                elif slot == 15:
                    nc.sync.dma_start(rb_full[112:128, :], r_dram[112:128, :])
                else:
                    nc.sync.dma_start(rb_full[hlf, :], r_dram[hlf, :])
                for j in range(3):  # d2x, d2y, d2z
                    def rbs(idx):
                        return rb_full[hlf, (idx - 1) * EPP:idx * EPP]
                    sq_t, lin_t = ep_t[2 * j], ep_t[2 * j + 1]
                    m1 = wp.tile([128, EPP], f32, tag="m1")
                    ee.tensor_mul(m1[hlf, :], sq_t[hlf, :], rbs(3 * j + 1))
                    m2 = wp.tile([128, EPP], f32, tag="m2")
                    ee.tensor_mul(m2[hlf, :], lin_t[hlf, :], rbs(3 * j + 2))
                    a1 = wp.tile([128, EPP], f32, tag="a1")
                    ee.tensor_add(a1[hlf, :], m1[hlf, :], m2[hlf, :])
                    d2 = wp.tile([128, EPP], f32, tag="d2")
                    ee.tensor_add(d2[hlf, :], a1[hlf, :], rbs(3 * j + 3))
                    nc.sync.dma_start(
                        out_d[j + 1:j + 2, psl].rearrange(
                            "o (p f) -> (o p) f", p=npart),
                        d2[hlf, :])
    nc.compile()
    return nc


def _get_program():
    if "nc" not in _COMPILED:
        _COMPILED["nc"] = _build_program()
    return _COMPILED["nc"]


def _host_prep(x, y, z, wx, bx, wy, by, wz, bz, coeff):
    """Build per-core input maps (features + coefficient matrices)."""
    f8 = np.float64
    wx64, bx64 = wx.astype(f8), bx.astype(f8)
    wy64, by64 = wy.astype(f8), by.astype(f8)
    wz64, bz64 = wz.astype(f8), bz.astype(f8)
    c64 = coeff.astype(f8)
    sc = np.sqrt(np.clip(wx64 * wy64 * wz64, 1e-12, None))
    Z = np.zeros_like(wx64)

    # s = xt^2 + yt^2 + zt^2 over features [x2,y2,z2,xyz,xy,xz,yz,x,y,z,1]
    Ls = np.stack([
        wx64 ** 2, wy64 ** 2, wz64 ** 2, Z, Z, Z, Z,
        -2 * wx64 * bx64, -2 * wy64 * by64, -2 * wz64 * bz64,
        bx64 ** 2 + by64 ** 2 + bz64 ** 2,
    ]).astype(np.float32)                      # [11, K]
    # T3 = xt*yt*zt
    Lt = np.stack([
        Z, Z, Z,
        wx64 * wy64 * wz64, -wx64 * wy64 * bz64, -wx64 * by64 * wz64,
        -bx64 * wy64 * wz64, wx64 * by64 * bz64, bx64 * wy64 * bz64,
        bx64 * by64 * wz64, -bx64 * by64 * bz64,
    ]).astype(np.float32)                      # [11, K]
    b1 = c64 * sc * wx64 ** 2
    b2 = c64 * sc * wy64 ** 2
    b3 = c64 * sc * wz64 ** 2
    Lo = np.stack([
        -c64 * sc,
        -b1 * wx64 ** 2, 2 * b1 * wx64 * bx64, b1 * (3 - bx64 ** 2),
        -b2 * wy64 ** 2, 2 * b2 * wy64 * by64, b2 * (3 - by64 ** 2),
        -b3 * wz64 ** 2, 2 * b3 * wz64 * bz64, b3 * (3 - bz64 ** 2),
    ], axis=1).astype(np.float32)              # [K, 10]

    Ls32 = _stack32(Ls)                        # [32, K] fp16
    Lt32 = _stack32(Lt)
    lst_pack = np.concatenate([Ls32, Lt32, Ls32, Lt32], axis=0)  # [128, K]
    Loh = Lo.astype(np.float16)
    loh_pack = np.concatenate(
        [Loh[kb * 128:(kb + 1) * 128] for kb in range(KBLK)], axis=1)  # [128, 40]

    in_maps = []
    for cid in range(N_CORES):
        sl = slice(cid * NP_CORE, (cid + 1) * NP_CORE)
        xs, ys, zs = (np.ascontiguousarray(a[sl], np.float32) for a in (x, y, z))
        F = np.stack([
            xs * xs, ys * ys, zs * zs, xs * ys * zs, xs * ys, xs * zs,
            ys * zs, xs, ys, zs, np.ones_like(xs),
        ]).astype(np.float32)                  # [11, NP_CORE]
        Fh, Fl = _split16(F)
        fst1 = np.concatenate([Fh, Fh, Fl[:NFEAT - 1]], axis=0)   # [32, NP]
        fst = np.concatenate([fst1, fst1, fst1, fst1], axis=0)    # [128, NP]
        ep = np.stack([xs * xs, xs, ys * ys, ys, zs * zs, zs]).astype(np.float32)
        in_maps.append({
            "fst": fst, "lst": lst_pack, "loh": loh_pack, "ep": ep,
        })
    return in_maps


def _run_device(in_maps, trace=False):
    from concourse.bass_utils import run_bass_kernel_spmd
    nc = _get_program()
    last_err = None
    for _attempt in range(3):
        try:
            return run_bass_kernel_spmd(
                nc, in_maps, list(range(N_CORES)), trace=trace)
        except Exception as ex:  # transient NRT device errors recover on retry
            last_err = ex
    raise last_err


def kernel(x, y, z, wx, bx, wy, by, wz, bz, coeff, bias, _trace=False):
    x, y, z = (np.asarray(a, np.float32) for a in (x, y, z))
    in_maps = _host_prep(
        x, y, z,
        *(np.asarray(a, np.float32) for a in (wx, bx, wy, by, wz, bz, coeff)))
    res = _run_device(in_maps, trace=_trace)
    outs = [res.results[cid]["out"] for cid in range(N_CORES)]
    full = np.concatenate(outs, axis=1)        # [4, N_TOTAL]
    bias_f = np.float32(np.asarray(bias))
    output = (full[0] + bias_f).astype(np.float32)
    if _trace:
        kernel._last_results = res
    return (output, full[1].copy(), full[2].copy(), full[3].copy())


# revision 11
# speedup vs baseline: 1.3094x; 1.3094x over previous
"""AWPINN wavelet-PINN kernel for 8x Trainium2 NeuronCores (Bass/Tile).

Math: for each point i and wavelet k (N=65536, K=512):
  xt = wx*x - bx (same y,z);  s = xt^2+yt^2+zt^2;  E = exp(-0.5*s)
  W  = xt*yt*zt*E          (reference's xw*yw*zw = -W)
  output = sum_k (-coeff*scale)_k * W + bias
  d2u_dx2 = sum_k (coeff*scale*wx^2)_k * (3 - xt^2) * W   (same y,z)

Device structure (v4, from the v1 pipeline):
  - s and T3=xt*yt*zt are low-rank bilinear forms in per-point features
    F = [x2,y2,z2,xyz,xy,xz,yz,x,y,z,1] -> TensorEngine matmuls
    (contraction = features, M = 128 wavelets/block, FD = 512 points).
    All matmuls fp16; near-fp32 via hi/lo split stacked along the
    contraction dim. The feature stack is replicated host-side to
    [128, NP] so each slice loads with ONE dma into all 4 row groups.
  - Output matmuls use hi-only fp16 weights (costs ~1e-3 rel err vs the
    2e-2 gate): kb0/kb1 accumulate into po_a (col group q0), kb2/kb3
    into po_b (q32), so consecutive matmuls overlap; ACT+DVE merge the
    two partials during the drain (same drain as v1, half the matmuls).
  - W = T3*E runs as ONE [128,1024] DVE op per k-pair (pspt holds
    [128,1024] pair tiles, bufs=2 -> no added serialization).
  - R bounces through DRAM point-major for the epilogue transpose; the
    output row skips the bounce (direct [1,n] dma from r_rows).
Data parallel over points: each core handles 8192 points; no collectives.
"""

import numpy as np

N_TOTAL = 65536
K_TOTAL = 512
N_CORES = 8
NP_CORE = N_TOTAL // N_CORES        # 8192 points per core
CHUNK = 512                         # points per matmul (PSUM bank = 512 fp32)
N_CHUNKS = NP_CORE // CHUNK         # 16
KBLK = K_TOTAL // 128               # 4 wavelet blocks of 128
EPP = NP_CORE // 128                # 64 = free dim of [128, 64] point layout
NFEAT = 11                          # features per point
NST = 32                            # stacked contraction rows (ones-lo dropped)

_COMPILED = {}


def _split16(a):
    """Split fp32 into fp16 hi + fp16 lo (hi+lo carries ~21 mantissa bits)."""
    a = np.ascontiguousarray(a, np.float32)
    hi = a.astype(np.float16)
    lo = np.float32(a - hi.astype(np.float32)).astype(np.float16)
    return hi, lo


def _stack32(L):
    """[11,n] fp32 coeffs -> [32,n] fp16 stack [Lh; Ll; Lh[:10]]."""
    Lh, Ll = _split16(L)
    return np.concatenate([Lh, Ll, Lh[:NFEAT - 1]], axis=0)


def _build_program():
    import concourse.bacc as bacc
    import concourse.mybir as mybir
    import concourse.tile as tile

    f32 = mybir.dt.float32
    f16 = mybir.dt.float16
    AF = mybir.ActivationFunctionType

    nc = bacc.Bacc("TRN2", target_bir_lowering=False, debug=False)

    # fst: feature stack, host-replicated across the 4 row groups
    fst_d = nc.dram_tensor("fst", [128, NP_CORE], f16, kind="ExternalInput")
    # lst: rows 0-31 Ls-stack, 32-63 Lt-stack, 64-95 Ls, 96-127 Lt;
    # columns grouped by k-block
    lst_d = nc.dram_tensor("lst", [128, K_TOTAL], f16, kind="ExternalInput")
    loh_d = nc.dram_tensor("loh", [128, KBLK * 10], f16, kind="ExternalInput")
    ep_d = nc.dram_tensor("ep", [6, NP_CORE], f32, kind="ExternalInput")
    out_d = nc.dram_tensor("out", [4, NP_CORE], f32, kind="ExternalOutput")

    with tile.TileContext(nc) as tc:
        with (
            tc.tile_pool(name="persist", bufs=1) as pp,
            tc.tile_pool(name="work", bufs=4) as wp,
            tc.tile_pool(name="psum_s", bufs=1, space="PSUM") as psps,
            tc.tile_pool(name="psum_t", bufs=2, space="PSUM") as pspt,
            tc.tile_pool(name="psum_out", bufs=2, space="PSUM") as pso,
            tc.tile_pool(name="wpool", bufs=8) as wpool,
            tc.tile_pool(name="dram", bufs=1, space="DRAM") as dp,
        ):
            lst_t = pp.tile([128, K_TOTAL], f16, tag="lst")
            nc.sync.dma_start(lst_t[:], lst_d[:])

            loh_t = pp.tile([128, KBLK * 10], f16, tag="loh")
            nc.scalar.dma_start(loh_t[:], loh_d[:])

            # persistent feature stack (host-replicated); slice 1 rides the
            # scalar queue so the first two land in parallel
            f_all = pp.tile([128, NP_CORE], f16, tag="f_all")
            bounds = [0, CHUNK, 2 * CHUNK] + [
                q * NP_CORE // 8 for q in range(2, 9)]
            for q in range(len(bounds) - 1):
                qs = slice(bounds[q], bounds[q + 1])
                eng = nc.scalar if q == 1 else nc.sync
                eng.dma_start(f_all[:, qs], fst_d[:, qs])

            r_rows = pp.tile([10, NP_CORE], f32, tag="r_rows")
            # R rows 1-9 in DRAM, point-major: r_dram[p, (r-1)*64 + f]
            r_dram = dp.tile([128, 9 * EPP], f32, tag="r_dram")

            ep_t = []
            for i in range(6):  # x2, x, y2, y, z2, z
                t = pp.tile([128, EPP], f32, tag=f"ep{i}")
                nc.gpsimd.dma_start(
                    t[:], ep_d[i:i + 1, :].rearrange("o (p f) -> (o p) f", p=128))
                ep_t.append(t)

            pending_drain = []

            def emit_drain():
                if not pending_drain:
                    return
                dc, dpo_a, dpo_b = pending_drain.pop()
                dst = r_rows[:, dc * CHUNK:(dc + 1) * CHUNK]
                nc.scalar.copy(dst, dpo_a)
                nc.vector.scalar_tensor_tensor(
                    dst, dpo_b, 1.0, dst,
                    mybir.AluOpType.mult, mybir.AluOpType.add)
                nc.sync.dma_start(
                    out_d[0:1, dc * CHUNK:(dc + 1) * CHUNK], dst[0:1, :])
                nc.sync.dma_start(
                    r_dram[dc * 8:(dc + 1) * 8, :].rearrange(
                        "p (r f) -> r p f", r=9),
                    dst[1:10, :].rearrange("r (p f) -> r p f", p=8))

            for c in range(N_CHUNKS):
                f_t = f_all[:, c * CHUNK:(c + 1) * CHUNK]
                po_ab = pso.tile([42, CHUNK], f32, tag="po_ab")
                po_a, po_b = po_ab[0:10, :], po_ab[32:42, :]
                w_ts = []
                for p in range(KBLK // 2):      # k-block pairs
                    kb0, kb1 = 2 * p, 2 * p + 1
                    ps_s = psps.tile([128, 2 * CHUNK], f32, tag="ps_s")
                    ps_t = pspt.tile([128, 2 * CHUNK], f32, tag="ps_t")
                    # 4 feature matmuls burst over the four 32-row groups
                    for g, (dst, kb) in [
                            (0, (ps_s, kb0)), (2, (ps_s, kb1)),
                            (1, (ps_t, kb0)), (3, (ps_t, kb1))]:
                        half = slice((kb & 1) * CHUNK, ((kb & 1) + 1) * CHUNK)
                        nc.tensor.matmul(
                            dst[:, half],
                            lst_t[32 * g:32 * (g + 1), kb * 128:(kb + 1) * 128],
                            f_t[32 * g:32 * (g + 1), :],
                            start=True, stop=True,
                            tile_position=(32 * g, 0))
                    e_t = wp.tile([128, 2 * CHUNK], f32, tag="e")
                    nc.scalar.activation(e_t[:], ps_s[:], AF.Exp, scale=-0.5)
                    w_t = wpool.tile([128, 2 * CHUNK], f16, tag="w")
                    nc.vector.tensor_mul(w_t[:], ps_t[:], e_t[:])
                    w_ts.append(w_t)
                    if p == 0:
                        emit_drain()   # previous chunk's R drain, off the
                                       # critical path of this chunk's E/W
                # output matmuls, hi-only weights: kb0/kb1 accumulate into
                # po_a (col group q0) while kb2/kb3 accumulate into po_b
                # (q32), so consecutive matmuls overlap
                for kb in range(KBLK):
                    w_t = w_ts[kb // 2]
                    half = slice((kb & 1) * CHUNK, ((kb & 1) + 1) * CHUNK)
                    dst = po_a if kb < 2 else po_b
                    nc.tensor.matmul(
                        dst, loh_t[:, kb * 10:(kb + 1) * 10], w_t[:, half],
                        start=(kb % 2 == 0), stop=(kb % 2 == 1))
                # defer this chunk's R drain so it does not delay the next
                # chunk's E/W in the ACT/DVE queues
                pending_drain.append((c, po_a, po_b))

                # two-phase epilogue: once half the points are in r_dram,
                # rebuild d2 outputs for those points. Point i lives at
                # [i // EPP, i % EPP] in the [128, EPP] layout, so a
                # point-half is a partition-half.
                if c not in (N_CHUNKS // 2 - 1, N_CHUNKS - 1):
                    continue
                emit_drain()
                hlf = slice(0, 64) if c == N_CHUNKS // 2 - 1 else slice(64, 128)
                psl = slice((hlf.start // 64) * NP_CORE // 2,
                            (hlf.start // 64 + 1) * NP_CORE // 2)
                # R1..R9 for this half, split across two dmas
                rb = wp.tile([128, 9 * EPP], f32, tag="rb")
                h0 = slice(hlf.start, hlf.start + 32)
                h1 = slice(hlf.start + 32, hlf.stop)
                nc.sync.dma_start(rb[h0, :], r_dram[h0, :])
                nc.sync.dma_start(rb[h1, :], r_dram[h1, :])
                # first half runs mid-loop on the idle GPSIMD; the final
                # half runs in the tail where DVE is idle and 2x faster
                ee = nc.gpsimd if hlf.start == 0 else nc.vector
                for j in range(3):  # d2x, d2y, d2z
                    def rbs(idx):
                        return rb[hlf, (idx - 1) * EPP:idx * EPP]
                    sq_t, lin_t = ep_t[2 * j], ep_t[2 * j + 1]
                    m1 = wp.tile([128, EPP], f32, tag="m1")
                    ee.tensor_mul(m1[hlf, :], sq_t[hlf, :], rbs(3 * j + 1))
                    m2 = wp.tile([128, EPP], f32, tag="m2")
                    ee.tensor_mul(m2[hlf, :], lin_t[hlf, :], rbs(3 * j + 2))
                    a1 = wp.tile([128, EPP], f32, tag="a1")
                    ee.tensor_add(a1[hlf, :], m1[hlf, :], m2[hlf, :])
                    d2 = wp.tile([128, EPP], f32, tag="d2")
                    ee.tensor_add(d2[hlf, :], a1[hlf, :], rbs(3 * j + 3))
                    nc.sync.dma_start(
                        out_d[j + 1:j + 2, psl].rearrange(
                            "o (p f) -> (o p) f", p=64),
                        d2[hlf, :])
    nc.compile()
    return nc


def _get_program():
    if "nc" not in _COMPILED:
        _COMPILED["nc"] = _build_program()
    return _COMPILED["nc"]


def _host_prep(x, y, z, wx, bx, wy, by, wz, bz, coeff):
    """Build per-core input maps (features + coefficient matrices)."""
    f8 = np.float64
    wx64, bx64 = wx.astype(f8), bx.astype(f8)
    wy64, by64 = wy.astype(f8), by.astype(f8)
    wz64, bz64 = wz.astype(f8), bz.astype(f8)
    c64 = coeff.astype(f8)
    sc = np.sqrt(np.clip(wx64 * wy64 * wz64, 1e-12, None))
    Z = np.zeros_like(wx64)

    # s = xt^2 + yt^2 + zt^2 over features [x2,y2,z2,xyz,xy,xz,yz,x,y,z,1]
    Ls = np.stack([
        wx64 ** 2, wy64 ** 2, wz64 ** 2, Z, Z, Z, Z,
        -2 * wx64 * bx64, -2 * wy64 * by64, -2 * wz64 * bz64,
        bx64 ** 2 + by64 ** 2 + bz64 ** 2,
    ]).astype(np.float32)                      # [11, K]
    # T3 = xt*yt*zt
    Lt = np.stack([
        Z, Z, Z,
        wx64 * wy64 * wz64, -wx64 * wy64 * bz64, -wx64 * by64 * wz64,
        -bx64 * wy64 * wz64, wx64 * by64 * bz64, bx64 * wy64 * bz64,
        bx64 * by64 * wz64, -bx64 * by64 * bz64,
    ]).astype(np.float32)                      # [11, K]
    b1 = c64 * sc * wx64 ** 2
    b2 = c64 * sc * wy64 ** 2
    b3 = c64 * sc * wz64 ** 2
    Lo = np.stack([
        -c64 * sc,
        -b1 * wx64 ** 2, 2 * b1 * wx64 * bx64, b1 * (3 - bx64 ** 2),
        -b2 * wy64 ** 2, 2 * b2 * wy64 * by64, b2 * (3 - by64 ** 2),
        -b3 * wz64 ** 2, 2 * b3 * wz64 * bz64, b3 * (3 - bz64 ** 2),
    ], axis=1).astype(np.float32)              # [K, 10]

    Ls32 = _stack32(Ls)                        # [32, K] fp16
    Lt32 = _stack32(Lt)
    lst_pack = np.concatenate([Ls32, Lt32, Ls32, Lt32], axis=0)  # [128, K]
    Loh = Lo.astype(np.float16)
    loh_pack = np.concatenate(
        [Loh[kb * 128:(kb + 1) * 128] for kb in range(KBLK)], axis=1)  # [128, 40]

    in_maps = []
    for cid in range(N_CORES):
        sl = slice(cid * NP_CORE, (cid + 1) * NP_CORE)
        xs, ys, zs = (np.ascontiguousarray(a[sl], np.float32) for a in (x, y, z))
        F = np.stack([
            xs * xs, ys * ys, zs * zs, xs * ys * zs, xs * ys, xs * zs,
            ys * zs, xs, ys, zs, np.ones_like(xs),
        ]).astype(np.float32)                  # [11, NP_CORE]
        Fh, Fl = _split16(F)
        fst1 = np.concatenate([Fh, Fh, Fl[:NFEAT - 1]], axis=0)   # [32, NP]
        fst = np.concatenate([fst1, fst1, fst1, fst1], axis=0)    # [128, NP]
        ep = np.stack([xs * xs, xs, ys * ys, ys, zs * zs, zs]).astype(np.float32)
        in_maps.append({
            "fst": fst, "lst": lst_pack, "loh": loh_pack, "ep": ep,
        })
    return in_maps


def _run_device(in_maps, trace=False):
    from concourse.bass_utils import run_bass_kernel_spmd
    nc = _get_program()
    last_err = None
    for _attempt in range(3):
        try:
            return run_bass_kernel_spmd(
                nc, in_maps, list(range(N_CORES)), trace=trace)
        except Exception as ex:  # transient NRT device errors recover on retry
            last_err = ex
    raise last_err


def kernel(x, y, z, wx, bx, wy, by, wz, bz, coeff, bias, _trace=False):
    x, y, z = (np.asarray(a, np.float32) for a in (x, y, z))
    in_maps = _host_prep(
        x, y, z,
        *(np.asarray(a, np.float32) for a in (wx, bx, wy, by, wz, bz, coeff)))
    res = _run_device(in_maps, trace=_trace)
    outs = [res.results[cid]["out"] for cid in range(N_CORES)]
    full = np.concatenate(outs, axis=1)        # [4, N_TOTAL]
    bias_f = np.float32(np.asarray(bias))
    output = (full[0] + bias_f).astype(np.float32)
    if _trace:
        kernel._last_results = res
    return (output, full[1].copy(), full[2].copy(), full[3].copy())

